# revision 21
# baseline (speedup 1.0000x reference)
# MoE layer (8 experts, top-2) on 8 TRN2 NeuronCores.
#
# Strategy: expert parallelism (core e owns expert e), per the sharding hint.
#   * Host (control plane): computes gate routing decisions, dispatches
#     ("all-to-all") each token's row to the core(s) owning its top-2 experts,
#     and combines the per-expert partial outputs back into the full output.
#   * Device (data plane): for each core e, computes
#         yT = sigmoid(dlg) * ( W2[e].T @ gelu( W1[e].T @ xT ) )
#     where xT is the (C x CAP) gathered token block for expert e (transposed
#     so the contraction dim lives on SBUF partitions), and sigmoid(dlg) is
#     exactly the top-2 softmax weight for the owning expert
#     (softmax([a,b])[0] == sigmoid(a-b)).
#
# Precision/throughput: both matmuls run on the PE array in fp8-e4m3 with
# MatmulPerfMode.DoubleRow (two stationary rows per PE cell, two 128-deep
# contraction slices per instruction at 0.5 cycles per output row => 4x bf16
# MAC throughput). Plain fp8 rounding is too coarse for the 2e-2 gate, so
# token-expert pairs are split into two classes:
#   * FULL: every operand is carried as a hi + lo fp8 pair (lo = fp8 of the
#     quantization residual) and each product is computed as
#     hi.hi + lo.hi + hi.lo (lo.lo dropped) -- 0.75x the bf16 PE cost,
#     rel-err 1.8e-3 on its own.
#   * CHEAP: plain fp8 (hi.hi only) -- 0.25x the bf16 PE cost. The ~5%
#     per-pair fp8 error is scaled by a small combine weight.
# Every core gets the SAME full-class size FSTAR: each expert's pairs are
# sorted by combine weight and the (n_e - FSTAR) smallest-weight ones go
# cheap. This balances the dominant full-class work across cores exactly and
# spends the error budget on the least-damaging pairs. Measured end-to-end
# rel-err 1.35e-2 (gate 2e-2; bf16 baseline is 3.4e-3).
#
# Layout (contraction dim on partitions, tokens on the moving free dim):
#   xh/xl   dram [P, CO, capF]   fp8, values 8*x   (full-class tokens)
#   xc      dram [P, CO, capC]   fp8, values 8*x   (cheap-class tokens)
#   w1h/w1l dram [P, CO, F]      fp8, values 64*W1
#   w2h/w2l dram [P, CO, FO, P]  fp8, values 256*W2
#   phase 1:  ps1 = 512*(W1.T @ x)  -> g = gelu(ps1/512)
#             full: g [ACT fp32], hh = fp8(g), hl = fp8(g - hh) [ACT/DVE
#             alternating by fo parity -- either engine alone saturates]
#             cheap: hh = fp8(gelu(ps1/512)), one ACT op per PAIR of psum
#             groups (PSUM tiles span 2 banks) -- per-instruction overhead
#             would otherwise make ACT the cheap-block bottleneck
#   phase 2:  ps2 = 256*(W2.T @ h)  -> y = ps2 * (sigmoid(dlg)/256)
# W1 (hi+lo) and W2-hi stay resident in SBUF; W2-lo streams per full token
# block; tokens stream in blocks of 512 (the PSUM-bank moving-dim limit for
# fp32). Block order: full blocks, cheap blocks, then one full-size full
# block LAST so the end-of-kernel drain overlaps long psum groups instead of
# the short cheap ones. Phase-2 groups defer the products touching the last
# fo pairs (whose hh/hl land latest) to the end of the group, hiding the
# phase-1 ACT/DVE tail.

import math

import numpy as np
import ml_dtypes

import concourse.bass as bass
import concourse.mybir as mybir
import concourse.tile as tile
from concourse import bacc
from concourse.bass_utils import run_bass_kernel_spmd

C = 1024          # d_model
F = 4096          # d_ff
E = 8             # experts == cores
P = 128           # SBUF partitions
NTOK = 512        # moving-dim token block (one PSUM bank of fp32)
CO = C // P       # 8 contraction chunks, phase 1
FO = F // P       # 32 contraction chunks, phase 2
SX = 8.0          # fp8 scale on x
SW1 = 64.0        # fp8 scale on W1   (psum1 = SX*SW1 * z = 512 z)
SW2 = 256.0       # fp8 scale on W2   (h at scale 1; psum2 = 256 * (h@W2))
FSTAR = 3314      # per-core full-class size; (n_e - FSTAR) smallest-weight
                  # pairs of each expert take the cheap path
FP8 = mybir.dt.float8e4
F32 = mybir.dt.float32
DR = mybir.MatmulPerfMode.DoubleRow
E4M3 = ml_dtypes.float8_e4m3fn

# Filled by kernel() on each call, for the test harness to inspect.
last_run_info: dict = {}

# NEFF-module memo: (capF, capC) -> compiled Bass module (routing is
# deterministic in the inputs, so repeat calls reuse the same module).
_nc_cache: dict = {}


def _build_ffn(caps, ntok: int = NTOK) -> bass.Bass:
    """Per-core expert-FFN kernel, fp8 DoubleRow with hi/lo error correction
    for full-class tokens and plain fp8 for cheap-class tokens."""
    capF, capC = caps
    capC_d = max(capC, 4)
    act_fn = mybir.ActivationFunctionType.Gelu
    nc = bacc.Bacc()

    xh_d = nc.dram_tensor("xh", [P, CO, capF], FP8, kind="ExternalInput")
    xl_d = nc.dram_tensor("xl", [P, CO, capF], FP8, kind="ExternalInput")
    xc_d = nc.dram_tensor("xc", [P, CO, capC_d], FP8, kind="ExternalInput")
    w1h_d = nc.dram_tensor("w1h", [P, CO, F], FP8, kind="ExternalInput")
    w1l_d = nc.dram_tensor("w1l", [P, CO, F], FP8, kind="ExternalInput")
    w2h_d = nc.dram_tensor("w2h", [P, CO, FO, P], FP8, kind="ExternalInput")
    w2l_d = nc.dram_tensor("w2l", [P, CO, FO, P], FP8, kind="ExternalInput")
    dlg_d = nc.dram_tensor("dlg", [P, capF], F32, kind="ExternalInput")
    dlgc_d = nc.dram_tensor("dlgc", [P, capC_d], F32, kind="ExternalInput")
    yt = nc.dram_tensor("yt", [C, capF], F32, kind="ExternalOutput")
    yt2 = nc.dram_tensor("yt2", [C, capC_d], F32, kind="ExternalOutput")

    yt_r = yt.rearrange("(co ci) t -> ci co t", ci=P)
    yt2_r = yt2.rearrange("(co ci) t -> ci co t", ci=P)

    with tile.TileContext(nc) as tc:
        with (
            tc.tile_pool(name="wts", bufs=1) as wpool,
            tc.tile_pool(name="w2s", bufs=2) as w2pool,
            tc.tile_pool(name="xts", bufs=2) as xpool,
            tc.tile_pool(name="hts", bufs=1) as hpool,
            tc.tile_pool(name="g32s", bufs=2) as gpool,
            tc.tile_pool(name="ces", bufs=2) as cepool,
            tc.tile_pool(name="yts", bufs=2) as ypool,
            tc.tile_pool(name="ps", bufs=4, space="PSUM") as pspool,
        ):
            # Block 0's token DMAs are interleaved with the first w1 chunks
            # in dependency order of the first psum group's matmuls (the DMA
            # queue is FIFO). Each w1 chunk is ONE strided DMA covering all
            # co (8 descriptors/partition): per-DMA DGE setup is ~0.6 us, so
            # fine-grained per-co transfers would serialize on the issuing
            # engine and starve the PE ramp.
            t0n = min(ntok, capF)
            w1h_sb = wpool.tile([P, CO, F], FP8, tag="w1h")
            w1l_sb = wpool.tile([P, CO, F], FP8, tag="w1l")
            xh0 = xpool.tile([P, CO, ntok], FP8, tag="xh")
            xl0 = xpool.tile([P, CO, ntok], FP8, tag="xl")
            nc.sync.dma_start(w1h_sb[:, :, 0:128], w1h_d[:, :, 0:128])
            nc.sync.dma_start(xh0[:, 0:4, :t0n], xh_d[:, 0:4, :t0n])
            nc.sync.dma_start(xh0[:, 4:CO, :t0n], xh_d[:, 4:CO, :t0n])
            nc.sync.dma_start(w1l_sb[:, :, 0:128], w1l_d[:, :, 0:128])
            nc.sync.dma_start(w1h_sb[:, :, 128:256], w1h_d[:, :, 128:256])
            nc.sync.dma_start(xl0[:, 0:4, :t0n], xl_d[:, 0:4, :t0n])
            nc.sync.dma_start(w1l_sb[:, :, 128:256], w1l_d[:, :, 128:256])
            nc.sync.dma_start(xl0[:, 4:CO, :t0n], xl_d[:, 4:CO, :t0n])
            f0 = 256
            for fch in (256, 512, 512, 1024, 1536):
                nc.sync.dma_start(w1h_sb[:, :, f0 : f0 + fch], w1h_d[:, :, f0 : f0 + fch])
                nc.sync.dma_start(w1l_sb[:, :, f0 : f0 + fch], w1l_d[:, :, f0 : f0 + fch])
                f0 += fch
            assert f0 == F
            w2h_sb = wpool.tile([P, CO, FO, P], FP8, tag="w2h")
            nc.sync.dma_start(w2h_sb[:, 0:4], w2h_d[:, 0:4])
            nc.sync.dma_start(w2h_sb[:, 4:CO], w2h_d[:, 4:CO])

            # Block schedule: full block 0 first (its x is already loading),
            # then the remaining full blocks except one full-size one, the
            # cheap blocks, and the reserved full-size full block last.
            nblkF = (capF + ntok - 1) // ntok
            nblkC = (capC + ntok - 1) // ntok
            fulls = [(False, b * ntok, min(ntok, capF - b * ntok))
                     for b in range(nblkF)]
            cheaps = [(True, b * ntok, min(ntok, capC - b * ntok))
                      for b in range(nblkC)]
            assert nblkF >= 2
            sched = [fulls[0]] + fulls[2:] + cheaps + [fulls[1]]

            for bi, (cheap, t0, tn) in enumerate(sched):
                x_src, dlg_src, y_dst = (
                    (xc_d, dlgc_d, yt2_r) if cheap else (xh_d, dlg_d, yt_r)
                )
                if bi == 0:
                    xh_t, xl_t = xh0, xl0
                else:
                    xh_t = xpool.tile([P, CO, ntok], FP8, tag="xh")
                    nc.sync.dma_start(xh_t[:, :, :tn], x_src[:, :, t0 : t0 + tn])
                    if not cheap:
                        xl_t = xpool.tile([P, CO, ntok], FP8, tag="xl")
                        nc.sync.dma_start(
                            xl_t[:, :, :tn], xl_d[:, :, t0 : t0 + tn]
                        )
                # Combine weight ce = sigmoid(dlg)/SW2, via
                # sigmoid(z) = 0.5*tanh(z/2) + 0.5 (tanh shares an ACT table
                # with gelu; sigmoid does not).
                dlg_t = cepool.tile([P, ntok], F32, tag="dlg", bufs=1)
                nc.sync.dma_start(dlg_t[:, :tn], dlg_src[:, t0 : t0 + tn])
                ce_t = cepool.tile([P, ntok], F32, tag="ce")
                nc.scalar.activation(
                    ce_t[:, :tn], dlg_t[:, :tn],
                    mybir.ActivationFunctionType.Tanh, scale=0.5,
                )
                nc.vector.tensor_scalar(
                    ce_t[:, :tn], ce_t[:, :tn], 0.5 / SW2, 0.5 / SW2,
                    mybir.AluOpType.mult, mybir.AluOpType.add,
                )

                # Phase 1: ps1 = 512*(W1.T @ x); g = gelu(ps1/512).
                # PSUM tiles span two banks = two consecutive fo groups.
                # Full blocks: hh = fp8(g), hl = fp8(g - hh), the three
                # elementwise ops alternating between ACT and DVE by fo
                # parity (each alone saturates and lags the PE into phase 2).
                # Cheap blocks: ONE ACT gelu per psum pair.
                # Mains are emitted first so block 0 can start on xh + the
                # first w1h chunk alone.
                hh_t = hpool.tile([P, FO, ntok], FP8, tag="hh")
                if not cheap:
                    hl_t = hpool.tile([P, FO, ntok], FP8, tag="hl")
                if cheap:
                    # Plain-fp8 phase 1: one 2-bank psum tile per fo PAIR and
                    # a single ACT gelu over both banks; per-instruction ACT
                    # overhead would otherwise outrun the 4-matmul groups.
                    for fo2 in range(FO // 2):
                        psp = pspool.tile([P, 2, ntok], F32, tag="psc", bufs=2)
                        for half in range(2):
                            fo = 2 * fo2 + half
                            col = slice(fo * P, (fo + 1) * P)
                            for j in range(CO // 2):
                                cp = slice(2 * j, 2 * j + 2)
                                nc.tensor.matmul(
                                    psp[:, half, :tn], w1h_sb[:, cp, col],
                                    xh_t[:, cp, :tn],
                                    start=(j == 0), stop=(j == CO // 2 - 1),
                                    perf_mode=DR,
                                )
                        nc.scalar.activation(
                            hh_t[:, 2 * fo2 : 2 * fo2 + 2, :tn], psp[:, :, :tn],
                            act_fn, scale=1.0 / 512,
                        )
                else:
                    for fo in range(FO):
                        col = slice(fo * P, (fo + 1) * P)
                        ps = pspool.tile([P, ntok], F32, tag="ps")
                        for j in range(CO // 2):
                            cp = slice(2 * j, 2 * j + 2)
                            nc.tensor.matmul(
                                ps[:, :tn], w1h_sb[:, cp, col], xh_t[:, cp, :tn],
                                start=(j == 0), stop=False, perf_mode=DR,
                            )
                        for j in range(CO // 2):
                            cp = slice(2 * j, 2 * j + 2)
                            nc.tensor.matmul(
                                ps[:, :tn], w1l_sb[:, cp, col], xh_t[:, cp, :tn],
                                start=False, stop=False, perf_mode=DR,
                            )
                        for j in range(CO // 2):
                            cp = slice(2 * j, 2 * j + 2)
                            nc.tensor.matmul(
                                ps[:, :tn], w1h_sb[:, cp, col], xl_t[:, cp, :tn],
                                start=False, stop=(j == CO // 2 - 1),
                                perf_mode=DR,
                            )
                        g32 = gpool.tile([P, ntok], F32, tag="g32")
                        if fo % 2 == 0:
                            nc.scalar.activation(
                                hh_t[:, fo, :tn], ps[:, :tn], act_fn,
                                scale=1.0 / 512,
                            )
                            nc.scalar.activation(
                                g32[:, :tn], ps[:, :tn], act_fn, scale=1.0 / 512
                            )
                        else:
                            nc.scalar.activation(
                                g32[:, :tn], ps[:, :tn], act_fn, scale=1.0 / 512
                            )
                            nc.vector.tensor_scalar(
                                hh_t[:, fo, :tn], g32[:, :tn], 1.0, 0.0,
                                mybir.AluOpType.mult, mybir.AluOpType.add,
                            )
                        nc.vector.tensor_tensor(
                            hl_t[:, fo, :tn], g32[:, :tn], hh_t[:, fo, :tn],
                            mybir.AluOpType.subtract,
                        )

                # Phase 2: ps2 = 256*(W2.T @ h); y = ps2 * ce. Products are
                # emitted round-robin per fo pair, with everything touching
                # the last two fo pairs (whose hh/hl land latest) deferred to
                # the very end of the group, hiding the phase-1 ACT/DVE tail.
                if cheap:
                    for co2 in range(CO // 2):
                        psA = pspool.tile([P, ntok], F32, tag="ps", name="psA")
                        psB = pspool.tile([P, ntok], F32, tag="ps", name="psB")
                        coA, coB = 2 * co2, 2 * co2 + 1
                        for j in range(FO // 2):
                            fp = slice(2 * j, 2 * j + 2)
                            nc.tensor.matmul(
                                psA[:, :tn], w2h_sb[:, coA, fp, :],
                                hh_t[:, fp, :tn],
                                start=(j == 0), stop=(j == FO // 2 - 1),
                                perf_mode=DR,
                            )
                            nc.tensor.matmul(
                                psB[:, :tn], w2h_sb[:, coB, fp, :],
                                hh_t[:, fp, :tn],
                                start=(j == 0), stop=(j == FO // 2 - 1),
                                perf_mode=DR,
                            )
                        for co, psx in ((coA, psA), (coB, psB)):
                            y_t = ypool.tile([P, ntok], F32, tag="y")
                            nc.vector.tensor_tensor(
                                y_t[:, :tn], psx[:, :tn], ce_t[:, :tn],
                                mybir.AluOpType.mult,
                            )
                            nc.sync.dma_start(
                                y_dst[:, co, t0 : t0 + tn], y_t[:, :tn]
                            )
                    continue
                for co in range(CO):
                    if cheap:
                        order = [(0, j) for j in range(FO // 2)]
                    else:
                        w2l_t = w2pool.tile([P, FO, P], FP8, tag="w2l")
                        nc.sync.dma_start(w2l_t[:], w2l_d[:, co])
                        late = FO // 2 - 2
                        order = []
                        for j in range(late):
                            order += [(0, j), (1, j), (2, j)]
                        order += [(0, late), (1, late), (0, late + 1),
                                  (1, late + 1), (2, late), (2, late + 1)]
                    ps2 = pspool.tile([P, ntok], F32, tag="ps")
                    for i, (kind, j) in enumerate(order):
                        fp = slice(2 * j, 2 * j + 2)
                        if kind == 0:
                            lhs, rhs = w2h_sb[:, co, fp, :], hh_t[:, fp, :tn]
                        elif kind == 1:
                            lhs, rhs = w2l_t[:, fp, :], hh_t[:, fp, :tn]
                        else:
                            lhs, rhs = w2h_sb[:, co, fp, :], hl_t[:, fp, :tn]
                        nc.tensor.matmul(
                            ps2[:, :tn], lhs, rhs,
                            start=(i == 0), stop=(i == len(order) - 1),
                            perf_mode=DR,
                        )
                    y_t = ypool.tile([P, ntok], F32, tag="y")
                    nc.vector.tensor_tensor(
                        y_t[:, :tn], ps2[:, :tn], ce_t[:, :tn],
                        mybir.AluOpType.mult,
                    )
                    nc.sync.dma_start(y_dst[:, co, t0 : t0 + tn], y_t[:, :tn])

    # bacc passes: register allocation, and crucially generate_event_semaphores,
    # which splits multi-wait sync conditions (HW allows 1 wait per instruction).
    nc.compile()

    # Guard: the Tile allocator believes SBUF is 224 KiB/partition (the ISA
    # constant), but exceeding ~192 KiB crashes the TRN2 exec unit. Keep a
    # hard ceiling so overflows fail at build time, not on silicon.
    hw = 0
    for alloc in nc.to_json()["functions"][0]["allocations"]:
        for ml in alloc.get("memorylocations") or []:
            if ml.get("type") == "SB":
                hw = max(hw, ml["addr"] + ml["dims"][1])
    assert hw <= 184 * 1024, f"SBUF high-water {hw / 1024:.1f} KiB exceeds 184 KiB"
    return nc


def _gate_jax_cpu(xf: np.ndarray, Wg: np.ndarray):
    """Reproduce the reference's gate bit-exactly: fp32 matmul + lax.top_k
    on the jax CPU backend (including its tie-breaking). Falls back to a
    numpy gate (correct except possibly on exact fp32 knife-edge ties) if
    jax is unavailable."""
    try:
        import jax

        cpu = jax.devices("cpu")[0]
        with jax.default_device(cpu):
            logits = jax.device_put(xf, cpu) @ jax.device_put(Wg, cpu)
            tv, ti = jax.lax.top_k(logits, 2)
            return np.asarray(ti), np.asarray(tv)
    except Exception:
        logits = xf @ Wg
        part = np.argpartition(-logits, 1, axis=1)[:, :2]
        pv = np.take_along_axis(logits, part, axis=1)
        order = np.argsort(-pv, axis=1, kind="stable")
        ti = np.take_along_axis(part, order, axis=1)
        tv = np.take_along_axis(logits, ti, axis=1)
        return ti, tv


def _split8(v: np.ndarray):
    """fp8-e4m3 hi/lo decomposition: hi = q(v), lo = q(v - hi)."""
    hi = v.astype(E4M3)
    lo = (v - hi.astype(np.float32)).astype(E4M3)
    return hi, lo


def _pack_tokens(xf, sel):
    """Gather token rows and fold to [P, CO, n] with features on partitions."""
    n = len(sel)
    return (SX * xf[sel].T).reshape(CO, P, n).transpose(1, 0, 2)


def kernel(x, Wg, W1, W2):
    x = np.asarray(x, dtype=np.float32)
    Wg = np.asarray(Wg, dtype=np.float32)
    W1 = np.asarray(W1, dtype=np.float32)
    W2 = np.asarray(W2, dtype=np.float32)

    B, T, _ = x.shape
    N = B * T
    xf = x.reshape(N, C)

    # ---- Gate + routing (control plane) ----
    # Routing decisions are knife-edge sensitive: for this problem one token
    # has a 2.7e-7 gap between its 2nd and 3rd expert logits, smaller than
    # fp32 GEMM rounding differences between BLAS implementations. Compute
    # the gate with the same jax-on-CPU ops the reference uses so the top-2
    # selection matches it bit-for-bit.
    top2, tv = _gate_jax_cpu(xf, Wg)                        # (N, 2) ids / logits

    # Per expert: sort pairs by combine weight ascending; the smallest
    # (n_e - FSTAR) go cheap so every core has exactly FSTAR full pairs.
    classes = []   # per expert: (sel_full, dlg_full, sel_cheap, dlg_cheap)
    for e in range(E):
        sels, ds = [], []
        for k in (0, 1):
            sel = np.nonzero(top2[:, k] == e)[0]
            sels.append(sel)
            ds.append(tv[sel, k] - tv[sel, 1 - k])
        sel = np.concatenate(sels)
        d = np.concatenate(ds)
        o = np.argsort(d, kind="stable")   # ascending weight
        nc_e = max(0, len(sel) - FSTAR)
        cheap_idx, full_idx = o[:nc_e], o[nc_e:]
        classes.append((sel[full_idx], d[full_idx], sel[cheap_idx], d[cheap_idx]))

    countsF = [len(c[0]) for c in classes]
    countsC = [len(c[2]) for c in classes]
    # caps need no partition alignment — tokens are the free dim everywhere.
    # Round to mult of 4 so fp8 rows stay 4-byte aligned.
    capF = max(NTOK * 2, math.ceil(max(countsF) / 4) * 4)
    capC = math.ceil(max(countsC) / 4) * 4

    # ---- Token dispatch (all-to-all equivalent) ----
    in_maps = []
    for e in range(E):
        sel_f, d_f, sel_c, d_c = classes[e]

        xh = np.zeros((P, CO, capF), dtype=E4M3)
        xl = np.zeros((P, CO, capF), dtype=E4M3)
        gh, gl = _split8(_pack_tokens(xf, sel_f))
        xh[:, :, : len(sel_f)] = gh
        xl[:, :, : len(sel_f)] = gl
        xc = np.zeros((P, CO, max(capC, 4)), dtype=E4M3)
        xc[:, :, : len(sel_c)] = _pack_tokens(xf, sel_c).astype(E4M3)

        def dlg_arr(d, cap):
            a = np.full((cap,), -60.0, dtype=np.float32)
            a[: len(d)] = d
            return np.ascontiguousarray(
                np.broadcast_to(a[None, :], (P, cap)), dtype=np.float32
            )

        w1h, w1l = _split8((SW1 * W1[e]).reshape(CO, P, F).transpose(1, 0, 2))
        # [fo, fi, co, cc] -> [fi, co, fo, cc]
        v2 = (SW2 * W2[e]).reshape(FO, P, CO, P).transpose(1, 2, 0, 3)
        w2h, w2l = _split8(v2)
        in_maps.append(
            {
                "xh": xh, "xl": xl, "xc": xc,
                "w1h": np.ascontiguousarray(w1h),
                "w1l": np.ascontiguousarray(w1l),
                "w2h": np.ascontiguousarray(w2h),
                "w2l": np.ascontiguousarray(w2l),
                "dlg": dlg_arr(d_f, capF),
                "dlgc": dlg_arr(d_c, max(capC, 4)),
            }
        )

    # ---- Expert FFN on the 8 NeuronCores ----
    caps = (capF, capC)
    nc = _nc_cache.get(caps)
    if nc is None:
        nc = _nc_cache[caps] = _build_ffn(caps)
    res = run_bass_kernel_spmd(nc, in_maps, core_ids=list(range(E)))

    global last_run_info
    last_run_info = {
        "cap": caps,
        "counts": [countsF, countsC],
        "exec_time_ns": res.exec_time_ns,
        "mean_exec_time_ns": res.mean_exec_time_ns,
        "instructions_and_trace": res.instructions_and_trace,
        "profile_json": res.profile_json,
    }

    # ---- Combine (weighted scatter-add) ----
    out = np.zeros((N, C), dtype=np.float32)
    for e in range(E):
        sel_f, d_f, sel_c, d_c = classes[e]
        out[sel_f] += res.results[e]["yt"][:, : len(sel_f)].T
        if len(sel_c):
            out[sel_c] += res.results[e]["yt2"][:, : len(sel_c)].T
    return out.reshape(B, T, C)


# revision 23
# speedup vs baseline: 1.0010x; 1.0010x over previous
# MoE layer (8 experts, top-2) on 8 TRN2 NeuronCores.
#
# Strategy: expert parallelism (core e owns expert e), per the sharding hint.
#   * Host (control plane): computes gate routing decisions, dispatches
#     ("all-to-all") each token's row to the core(s) owning its top-2 experts,
#     and combines the per-expert partial outputs back into the full output.
#   * Device (data plane): for each core e, computes
#         yT = sigmoid(dlg) * ( W2[e].T @ gelu( W1[e].T @ xT ) )
#     where xT is the (C x CAP) gathered token block for expert e (transposed
#     so the contraction dim lives on SBUF partitions), and sigmoid(dlg) is
#     exactly the top-2 softmax weight for the owning expert
#     (softmax([a,b])[0] == sigmoid(a-b)).
#
# Precision/throughput: both matmuls run on the PE array in fp8-e4m3 with
# MatmulPerfMode.DoubleRow (two stationary rows per PE cell, two 128-deep
# contraction slices per instruction at 0.5 cycles per output row => 4x bf16
# MAC throughput). Plain fp8 rounding is too coarse for the 2e-2 gate, so
# token-expert pairs are split into two classes:
#   * FULL: every operand is carried as a hi + lo fp8 pair (lo = fp8 of the
#     quantization residual) and each product is computed as
#     hi.hi + lo.hi + hi.lo (lo.lo dropped) -- 0.75x the bf16 PE cost,
#     rel-err 1.8e-3 on its own.
#   * CHEAP: plain fp8 (hi.hi only) -- 0.25x the bf16 PE cost. The ~5%
#     per-pair fp8 error is scaled by a small combine weight.
# Every core gets the SAME full-class size FSTAR: each expert's pairs are
# sorted by combine weight and the (n_e - FSTAR) smallest-weight ones go
# cheap. This balances the dominant full-class work across cores exactly and
# spends the error budget on the least-damaging pairs. Measured end-to-end
# rel-err 1.35e-2 (gate 2e-2; bf16 baseline is 3.4e-3).
#
# Layout (contraction dim on partitions, tokens on the moving free dim):
#   xh/xl   dram [P, CO, capF]   fp8, values 8*x   (full-class tokens)
#   xc      dram [P, CO, capC]   fp8, values 8*x   (cheap-class tokens)
#   w1h/w1l dram [P, CO, F]      fp8, values 64*W1
#   w2h/w2l dram [P, CO, FO, P]  fp8, values 256*W2
#   phase 1:  ps1 = 512*(W1.T @ x)  -> g = gelu(ps1/512)
#             full: g [ACT fp32], hh = fp8(g), hl = fp8(g - hh) [ACT/DVE
#             alternating by fo parity -- either engine alone saturates]
#             cheap: hh = fp8(gelu(ps1/512)), one ACT op per PAIR of psum
#             groups (PSUM tiles span 2 banks) -- per-instruction overhead
#             would otherwise make ACT the cheap-block bottleneck
#   phase 2:  ps2 = 256*(W2.T @ h)  -> y = ps2 * (sigmoid(dlg)/256)
# W1 (hi+lo) and W2-hi stay resident in SBUF; W2-lo streams per full token
# block; tokens stream in blocks of 512 (the PSUM-bank moving-dim limit for
# fp32). Block order: full blocks, cheap blocks, then one full-size full
# block LAST so the end-of-kernel drain overlaps long psum groups instead of
# the short cheap ones. Phase-2 groups defer the products touching the last
# fo pairs (whose hh/hl land latest) to the end of the group, hiding the
# phase-1 ACT/DVE tail.

import math

import numpy as np
import ml_dtypes

import concourse.bass as bass
import concourse.mybir as mybir
import concourse.tile as tile
from concourse import bacc
from concourse.bass_utils import run_bass_kernel_spmd

C = 1024          # d_model
F = 4096          # d_ff
E = 8             # experts == cores
P = 128           # SBUF partitions
NTOK = 512        # moving-dim token block (one PSUM bank of fp32)
CO = C // P       # 8 contraction chunks, phase 1
FO = F // P       # 32 contraction chunks, phase 2
SX = 8.0          # fp8 scale on x
SW1 = 64.0        # fp8 scale on W1   (psum1 = SX*SW1 * z = 512 z)
SW2 = 256.0       # fp8 scale on W2   (h at scale 1; psum2 = 256 * (h@W2))
FSTAR = 3314      # per-core full-class size; (n_e - FSTAR) smallest-weight
                  # pairs of each expert take the cheap path
FP8 = mybir.dt.float8e4
F32 = mybir.dt.float32
DR = mybir.MatmulPerfMode.DoubleRow
E4M3 = ml_dtypes.float8_e4m3fn

# Filled by kernel() on each call, for the test harness to inspect.
last_run_info: dict = {}

# NEFF-module memo: (capF, capC) -> compiled Bass module (routing is
# deterministic in the inputs, so repeat calls reuse the same module).
_nc_cache: dict = {}


def _build_ffn(caps, ntok: int = NTOK) -> bass.Bass:
    """Per-core expert-FFN kernel, fp8 DoubleRow with hi/lo error correction
    for full-class tokens and plain fp8 for cheap-class tokens."""
    capF, capC = caps
    capC_d = max(capC, 4)
    act_fn = mybir.ActivationFunctionType.Gelu
    nc = bacc.Bacc()

    xh_d = nc.dram_tensor("xh", [P, CO, capF], FP8, kind="ExternalInput")
    xl_d = nc.dram_tensor("xl", [P, CO, capF], FP8, kind="ExternalInput")
    xc_d = nc.dram_tensor("xc", [P, CO, capC_d], FP8, kind="ExternalInput")
    w1h_d = nc.dram_tensor("w1h", [P, CO, F], FP8, kind="ExternalInput")
    w1l_d = nc.dram_tensor("w1l", [P, CO, F], FP8, kind="ExternalInput")
    w2h_d = nc.dram_tensor("w2h", [P, CO, FO, P], FP8, kind="ExternalInput")
    w2l_d = nc.dram_tensor("w2l", [P, CO, FO, P], FP8, kind="ExternalInput")
    dlg_d = nc.dram_tensor("dlg", [P, capF], F32, kind="ExternalInput")
    dlgc_d = nc.dram_tensor("dlgc", [P, capC_d], F32, kind="ExternalInput")
    yt = nc.dram_tensor("yt", [C, capF], F32, kind="ExternalOutput")
    yt2 = nc.dram_tensor("yt2", [C, capC_d], F32, kind="ExternalOutput")

    yt_r = yt.rearrange("(co ci) t -> ci co t", ci=P)
    yt2_r = yt2.rearrange("(co ci) t -> ci co t", ci=P)

    with tile.TileContext(nc) as tc:
        with (
            tc.tile_pool(name="wts", bufs=1) as wpool,
            tc.tile_pool(name="w2s", bufs=2) as w2pool,
            tc.tile_pool(name="xts", bufs=2) as xpool,
            tc.tile_pool(name="hts", bufs=1) as hpool,
            tc.tile_pool(name="g32s", bufs=2) as gpool,
            tc.tile_pool(name="ces", bufs=2) as cepool,
            tc.tile_pool(name="yts", bufs=2) as ypool,
            tc.tile_pool(name="ps", bufs=4, space="PSUM") as pspool,
        ):
            # Block 0's token DMAs are interleaved with the first w1 chunks
            # in dependency order of the first psum group's matmuls (the DMA
            # queue is FIFO). Each w1 chunk is ONE strided DMA covering all
            # co (8 descriptors/partition): per-DMA DGE setup is ~0.6 us, so
            # fine-grained per-co transfers would serialize on the issuing
            # engine and starve the PE ramp.
            t0n = min(ntok, capF)
            w1h_sb = wpool.tile([P, CO, F], FP8, tag="w1h")
            w1l_sb = wpool.tile([P, CO, F], FP8, tag="w1l")
            xh0 = xpool.tile([P, CO, ntok], FP8, tag="xh")
            xl0 = xpool.tile([P, CO, ntok], FP8, tag="xl")
            nc.sync.dma_start(w1h_sb[:, :, 0:128], w1h_d[:, :, 0:128])
            nc.sync.dma_start(xh0[:, 0:4, :t0n], xh_d[:, 0:4, :t0n])
            nc.sync.dma_start(xh0[:, 4:CO, :t0n], xh_d[:, 4:CO, :t0n])
            nc.sync.dma_start(w1l_sb[:, :, 0:128], w1l_d[:, :, 0:128])
            nc.sync.dma_start(xl0[:, 0:4, :t0n], xl_d[:, 0:4, :t0n])
            nc.sync.dma_start(xl0[:, 4:CO, :t0n], xl_d[:, 4:CO, :t0n])
            f0 = 128
            for fch in (128, 256, 512, 512, 1024, 1536):
                nc.sync.dma_start(w1h_sb[:, :, f0 : f0 + fch], w1h_d[:, :, f0 : f0 + fch])
                nc.sync.dma_start(w1l_sb[:, :, f0 : f0 + fch], w1l_d[:, :, f0 : f0 + fch])
                f0 += fch
            assert f0 == F
            w2h_sb = wpool.tile([P, CO, FO, P], FP8, tag="w2h")
            nc.sync.dma_start(w2h_sb[:], w2h_d[:])

            # Block schedule: full block 0 first (its x is already loading),
            # then the remaining full blocks except one full-size one, the
            # cheap blocks, and the reserved full-size full block last.
            nblkF = (capF + ntok - 1) // ntok
            nblkC = (capC + ntok - 1) // ntok
            fulls = [(False, b * ntok, min(ntok, capF - b * ntok))
                     for b in range(nblkF)]
            cheaps = [(True, b * ntok, min(ntok, capC - b * ntok))
                      for b in range(nblkC)]
            assert nblkF >= 2
            sched = [fulls[0]] + fulls[2:] + cheaps + [fulls[1]]

            for bi, (cheap, t0, tn) in enumerate(sched):
                x_src, dlg_src, y_dst = (
                    (xc_d, dlgc_d, yt2_r) if cheap else (xh_d, dlg_d, yt_r)
                )
                if bi == 0:
                    xh_t, xl_t = xh0, xl0
                else:
                    xh_t = xpool.tile([P, CO, ntok], FP8, tag="xh")
                    nc.sync.dma_start(xh_t[:, :, :tn], x_src[:, :, t0 : t0 + tn])
                    if not cheap:
                        xl_t = xpool.tile([P, CO, ntok], FP8, tag="xl")
                        nc.sync.dma_start(
                            xl_t[:, :, :tn], xl_d[:, :, t0 : t0 + tn]
                        )
                # Combine weight ce = sigmoid(dlg)/SW2, via
                # sigmoid(z) = 0.5*tanh(z/2) + 0.5 (tanh shares an ACT table
                # with gelu; sigmoid does not).
                dlg_t = cepool.tile([P, ntok], F32, tag="dlg", bufs=1)
                nc.sync.dma_start(dlg_t[:, :tn], dlg_src[:, t0 : t0 + tn])
                ce_t = cepool.tile([P, ntok], F32, tag="ce")
                nc.scalar.activation(
                    ce_t[:, :tn], dlg_t[:, :tn],
                    mybir.ActivationFunctionType.Tanh, scale=0.5,
                )
                nc.vector.tensor_scalar(
                    ce_t[:, :tn], ce_t[:, :tn], 0.5 / SW2, 0.5 / SW2,
                    mybir.AluOpType.mult, mybir.AluOpType.add,
                )

                # Phase 1: ps1 = 512*(W1.T @ x); g = gelu(ps1/512).
                # PSUM tiles span two banks = two consecutive fo groups.
                # Full blocks: hh = fp8(g), hl = fp8(g - hh), the three
                # elementwise ops alternating between ACT and DVE by fo
                # parity (each alone saturates and lags the PE into phase 2).
                # Cheap blocks: ONE ACT gelu per psum pair.
                # Mains are emitted first so block 0 can start on xh + the
                # first w1h chunk alone.
                hh_t = hpool.tile([P, FO, ntok], FP8, tag="hh")
                if not cheap:
                    hl_t = hpool.tile([P, FO, ntok], FP8, tag="hl")
                if cheap:
                    # Plain-fp8 phase 1: one 2-bank psum tile per fo PAIR and
                    # a single ACT gelu over both banks; per-instruction ACT
                    # overhead would otherwise outrun the 4-matmul groups.
                    for fo2 in range(FO // 2):
                        psp = pspool.tile([P, 2, ntok], F32, tag="psc", bufs=2)
                        for half in range(2):
                            fo = 2 * fo2 + half
                            col = slice(fo * P, (fo + 1) * P)
                            for j in range(CO // 2):
                                cp = slice(2 * j, 2 * j + 2)
                                nc.tensor.matmul(
                                    psp[:, half, :tn], w1h_sb[:, cp, col],
                                    xh_t[:, cp, :tn],
                                    start=(j == 0), stop=(j == CO // 2 - 1),
                                    perf_mode=DR,
                                )
                        nc.scalar.activation(
                            hh_t[:, 2 * fo2 : 2 * fo2 + 2, :tn], psp[:, :, :tn],
                            act_fn, scale=1.0 / 512,
                        )
                else:
                    for fo in range(FO):
                        col = slice(fo * P, (fo + 1) * P)
                        ps = pspool.tile([P, ntok], F32, tag="ps")
                        for j in range(CO // 2):
                            cp = slice(2 * j, 2 * j + 2)
                            nc.tensor.matmul(
                                ps[:, :tn], w1h_sb[:, cp, col], xh_t[:, cp, :tn],
                                start=(j == 0), stop=False, perf_mode=DR,
                            )
                        for j in range(CO // 2):
                            cp = slice(2 * j, 2 * j + 2)
                            nc.tensor.matmul(
                                ps[:, :tn], w1l_sb[:, cp, col], xh_t[:, cp, :tn],
                                start=False, stop=False, perf_mode=DR,
                            )
                        for j in range(CO // 2):
                            cp = slice(2 * j, 2 * j + 2)
                            nc.tensor.matmul(
                                ps[:, :tn], w1h_sb[:, cp, col], xl_t[:, cp, :tn],
                                start=False, stop=(j == CO // 2 - 1),
                                perf_mode=DR,
                            )
                        g32 = gpool.tile([P, ntok], F32, tag="g32")
                        if fo % 2 == 0:
                            nc.scalar.activation(
                                hh_t[:, fo, :tn], ps[:, :tn], act_fn,
                                scale=1.0 / 512,
                            )
                            nc.scalar.activation(
                                g32[:, :tn], ps[:, :tn], act_fn, scale=1.0 / 512
                            )
                        else:
                            nc.scalar.activation(
                                g32[:, :tn], ps[:, :tn], act_fn, scale=1.0 / 512
                            )
                            nc.vector.tensor_scalar(
                                hh_t[:, fo, :tn], g32[:, :tn], 1.0, 0.0,
                                mybir.AluOpType.mult, mybir.AluOpType.add,
                            )
                        nc.vector.tensor_tensor(
                            hl_t[:, fo, :tn], g32[:, :tn], hh_t[:, fo, :tn],
                            mybir.AluOpType.subtract,
                        )

                # Phase 2: ps2 = 256*(W2.T @ h); y = ps2 * ce. Products are
                # emitted round-robin per fo pair, with everything touching
                # the last two fo pairs (whose hh/hl land latest) deferred to
                # the very end of the group, hiding the phase-1 ACT/DVE tail.
                if cheap:
                    for co2 in range(CO // 2):
                        psA = pspool.tile([P, ntok], F32, tag="ps", name="psA")
                        psB = pspool.tile([P, ntok], F32, tag="ps", name="psB")
                        coA, coB = 2 * co2, 2 * co2 + 1
                        for j in range(FO // 2):
                            fp = slice(2 * j, 2 * j + 2)
                            nc.tensor.matmul(
                                psA[:, :tn], w2h_sb[:, coA, fp, :],
                                hh_t[:, fp, :tn],
                                start=(j == 0), stop=(j == FO // 2 - 1),
                                perf_mode=DR,
                            )
                            nc.tensor.matmul(
                                psB[:, :tn], w2h_sb[:, coB, fp, :],
                                hh_t[:, fp, :tn],
                                start=(j == 0), stop=(j == FO // 2 - 1),
                                perf_mode=DR,
                            )
                        for co, psx in ((coA, psA), (coB, psB)):
                            y_t = ypool.tile([P, ntok], F32, tag="y")
                            nc.vector.tensor_tensor(
                                y_t[:, :tn], psx[:, :tn], ce_t[:, :tn],
                                mybir.AluOpType.mult,
                            )
                            nc.sync.dma_start(
                                y_dst[:, co, t0 : t0 + tn], y_t[:, :tn]
                            )
                    continue
                for co in range(CO):
                    if cheap:
                        order = [(0, j) for j in range(FO // 2)]
                    else:
                        w2l_t = w2pool.tile([P, FO, P], FP8, tag="w2l")
                        nc.sync.dma_start(w2l_t[:], w2l_d[:, co])
                        late = FO // 2 - 2
                        order = []
                        for j in range(late):
                            order += [(0, j), (1, j), (2, j)]
                        order += [(0, late), (1, late), (0, late + 1),
                                  (1, late + 1), (2, late), (2, late + 1)]
                    # The very last psum group of the kernel is split into
                    # two token halves so the first half's y-mult + DMA
                    # overlap the second half's matmuls, trimming the
                    # end-of-kernel drain exposure.
                    last_grp = bi == len(sched) - 1 and co == CO - 1
                    halves = (
                        [(0, tn // 2), (tn // 2, tn)] if last_grp and tn >= 8
                        else [(0, tn)]
                    )
                    for ta, tb in halves:
                        ps2 = pspool.tile([P, ntok], F32, tag="ps")
                        for i, (kind, j) in enumerate(order):
                            fp = slice(2 * j, 2 * j + 2)
                            if kind == 0:
                                lhs, rhs = w2h_sb[:, co, fp, :], hh_t[:, fp, ta:tb]
                            elif kind == 1:
                                lhs, rhs = w2l_t[:, fp, :], hh_t[:, fp, ta:tb]
                            else:
                                lhs, rhs = w2h_sb[:, co, fp, :], hl_t[:, fp, ta:tb]
                            nc.tensor.matmul(
                                ps2[:, : tb - ta], lhs, rhs,
                                start=(i == 0), stop=(i == len(order) - 1),
                                perf_mode=DR,
                            )
                        y_t = ypool.tile([P, ntok], F32, tag="y")
                        nc.vector.tensor_tensor(
                            y_t[:, : tb - ta], ps2[:, : tb - ta],
                            ce_t[:, ta:tb], mybir.AluOpType.mult,
                        )
                        nc.sync.dma_start(
                            y_dst[:, co, t0 + ta : t0 + tb], y_t[:, : tb - ta]
                        )

    # bacc passes: register allocation, and crucially generate_event_semaphores,
    # which splits multi-wait sync conditions (HW allows 1 wait per instruction).
    nc.compile()

    # Guard: the Tile allocator believes SBUF is 224 KiB/partition (the ISA
    # constant), but exceeding ~192 KiB crashes the TRN2 exec unit. Keep a
    # hard ceiling so overflows fail at build time, not on silicon.
    hw = 0
    for alloc in nc.to_json()["functions"][0]["allocations"]:
        for ml in alloc.get("memorylocations") or []:
            if ml.get("type") == "SB":
                hw = max(hw, ml["addr"] + ml["dims"][1])
    assert hw <= 184 * 1024, f"SBUF high-water {hw / 1024:.1f} KiB exceeds 184 KiB"
    return nc


def _gate_jax_cpu(xf: np.ndarray, Wg: np.ndarray):
    """Reproduce the reference's gate bit-exactly: fp32 matmul + lax.top_k
    on the jax CPU backend (including its tie-breaking). Falls back to a
    numpy gate (correct except possibly on exact fp32 knife-edge ties) if
    jax is unavailable."""
    try:
        import jax

        cpu = jax.devices("cpu")[0]
        with jax.default_device(cpu):
            logits = jax.device_put(xf, cpu) @ jax.device_put(Wg, cpu)
            tv, ti = jax.lax.top_k(logits, 2)
            return np.asarray(ti), np.asarray(tv)
    except Exception:
        logits = xf @ Wg
        part = np.argpartition(-logits, 1, axis=1)[:, :2]
        pv = np.take_along_axis(logits, part, axis=1)
        order = np.argsort(-pv, axis=1, kind="stable")
        ti = np.take_along_axis(part, order, axis=1)
        tv = np.take_along_axis(logits, ti, axis=1)
        return ti, tv


def _split8(v: np.ndarray):
    """fp8-e4m3 hi/lo decomposition: hi = q(v), lo = q(v - hi)."""
    hi = v.astype(E4M3)
    lo = (v - hi.astype(np.float32)).astype(E4M3)
    return hi, lo


def _pack_tokens(xf, sel):
    """Gather token rows and fold to [P, CO, n] with features on partitions."""
    n = len(sel)
    return (SX * xf[sel].T).reshape(CO, P, n).transpose(1, 0, 2)


def kernel(x, Wg, W1, W2):
    x = np.asarray(x, dtype=np.float32)
    Wg = np.asarray(Wg, dtype=np.float32)
    W1 = np.asarray(W1, dtype=np.float32)
    W2 = np.asarray(W2, dtype=np.float32)

    B, T, _ = x.shape
    N = B * T
    xf = x.reshape(N, C)

    # ---- Gate + routing (control plane) ----
    # Routing decisions are knife-edge sensitive: for this problem one token
    # has a 2.7e-7 gap between its 2nd and 3rd expert logits, smaller than
    # fp32 GEMM rounding differences between BLAS implementations. Compute
    # the gate with the same jax-on-CPU ops the reference uses so the top-2
    # selection matches it bit-for-bit.
    top2, tv = _gate_jax_cpu(xf, Wg)                        # (N, 2) ids / logits

    # Per expert: sort pairs by combine weight ascending; the smallest
    # (n_e - FSTAR) go cheap so every core has exactly FSTAR full pairs.
    classes = []   # per expert: (sel_full, dlg_full, sel_cheap, dlg_cheap)
    for e in range(E):
        sels, ds = [], []
        for k in (0, 1):
            sel = np.nonzero(top2[:, k] == e)[0]
            sels.append(sel)
            ds.append(tv[sel, k] - tv[sel, 1 - k])
        sel = np.concatenate(sels)
        d = np.concatenate(ds)
        o = np.argsort(d, kind="stable")   # ascending weight
        nc_e = max(0, len(sel) - FSTAR)
        cheap_idx, full_idx = o[:nc_e], o[nc_e:]
        classes.append((sel[full_idx], d[full_idx], sel[cheap_idx], d[cheap_idx]))

    countsF = [len(c[0]) for c in classes]
    countsC = [len(c[2]) for c in classes]
    # caps need no partition alignment — tokens are the free dim everywhere.
    # Round to mult of 4 so fp8 rows stay 4-byte aligned.
    capF = max(NTOK * 2, math.ceil(max(countsF) / 4) * 4)
    capC = math.ceil(max(countsC) / 4) * 4

    # ---- Token dispatch (all-to-all equivalent) ----
    in_maps = []
    for e in range(E):
        sel_f, d_f, sel_c, d_c = classes[e]

        xh = np.zeros((P, CO, capF), dtype=E4M3)
        xl = np.zeros((P, CO, capF), dtype=E4M3)
        gh, gl = _split8(_pack_tokens(xf, sel_f))
        xh[:, :, : len(sel_f)] = gh
        xl[:, :, : len(sel_f)] = gl
        xc = np.zeros((P, CO, max(capC, 4)), dtype=E4M3)
        xc[:, :, : len(sel_c)] = _pack_tokens(xf, sel_c).astype(E4M3)

        def dlg_arr(d, cap):
            a = np.full((cap,), -60.0, dtype=np.float32)
            a[: len(d)] = d
            return np.ascontiguousarray(
                np.broadcast_to(a[None, :], (P, cap)), dtype=np.float32
            )

        w1h, w1l = _split8((SW1 * W1[e]).reshape(CO, P, F).transpose(1, 0, 2))
        # [fo, fi, co, cc] -> [fi, co, fo, cc]
        v2 = (SW2 * W2[e]).reshape(FO, P, CO, P).transpose(1, 2, 0, 3)
        w2h, w2l = _split8(v2)
        in_maps.append(
            {
                "xh": xh, "xl": xl, "xc": xc,
                "w1h": np.ascontiguousarray(w1h),
                "w1l": np.ascontiguousarray(w1l),
                "w2h": np.ascontiguousarray(w2h),
                "w2l": np.ascontiguousarray(w2l),
                "dlg": dlg_arr(d_f, capF),
                "dlgc": dlg_arr(d_c, max(capC, 4)),
            }
        )

    # ---- Expert FFN on the 8 NeuronCores ----
    caps = (capF, capC)
    nc = _nc_cache.get(caps)
    if nc is None:
        nc = _nc_cache[caps] = _build_ffn(caps)
    res = run_bass_kernel_spmd(nc, in_maps, core_ids=list(range(E)))

    global last_run_info
    last_run_info = {
        "cap": caps,
        "counts": [countsF, countsC],
        "exec_time_ns": res.exec_time_ns,
        "mean_exec_time_ns": res.mean_exec_time_ns,
        "instructions_and_trace": res.instructions_and_trace,
        "profile_json": res.profile_json,
    }

    # ---- Combine (weighted scatter-add) ----
    out = np.zeros((N, C), dtype=np.float32)
    for e in range(E):
        sel_f, d_f, sel_c, d_c = classes[e]
        out[sel_f] += res.results[e]["yt"][:, : len(sel_f)].T
        if len(sel_c):
            out[sel_c] += res.results[e]["yt2"][:, : len(sel_c)].T
    return out.reshape(B, T, C)
